# revision 2
# baseline (speedup 1.0000x reference)
"""Trainium2 Bass kernel for a DeepSeek-style MoE layer.

Module: 16 routed experts (top-2), 1 shared expert, SwiGLU FFNs.
  x: [4, 2048, 1024] -> y: [4, 2048, 1024], probs: [8192, 16]

Sharding strategy (expert-parallel, per the hint):
  - The router top-2 dispatch is computed on the host (fp64) as part of the
    sharding step: tokens are gathered per expert, padded to a fixed capacity
    CAP, and experts (2c, 2c+1) are assigned to core c.
  - Every tensor is staged TRANSPOSED ([D, T] / [D, F] layouts) so that all
    DMAs are contiguous and matmul contraction dims land on SBUF partitions.
  - Each core computes ON DEVICE: the router (logits + softmax -> probs) and
    the shared-expert FFN for its 1024-token shard (data-parallel), plus the
    full SwiGLU FFN of its 2 experts over their dispatched tokens, scaled by
    the combine weights (expert-parallel).
  - The host combine step adds shared + routed contributions back per token.

Numerics: bf16 matmul operands, fp32 PSUM accumulation, fp32 outputs.
"""

import numpy as np
import ml_dtypes

B, L, D, F, E = 4, 2048, 1024, 512, 16
T = B * L          # 8192 tokens
NCORES = 8
TSH = T // NCORES  # tokens per core for the data-parallel parts
NSLOT = 2          # experts per core

BF16 = np.dtype(ml_dtypes.bfloat16)

_CACHE = {}


# ----------------------------------------------------------------------------
# Host-side routing (fp64 mirror of the reference router; used for dispatch)
# ----------------------------------------------------------------------------

def _route(flat_x, gate_w, expert_bias):
    lg = flat_x.astype(np.float64) @ gate_w.T.astype(np.float64)
    lg += expert_bias.astype(np.float64)
    lg -= lg.max(-1, keepdims=True)
    p = np.exp(lg)
    p /= p.sum(-1, keepdims=True)
    t_idx = np.arange(T)
    i1 = np.argmax(p, axis=-1)
    pm = p.copy()
    pm[t_idx, i1] = -np.inf
    i2 = np.argmax(pm, axis=-1)
    w1 = p[t_idx, i1]
    w2 = p[t_idx, i2]
    s = w1 + w2
    return i1, i2, (w1 / s).astype(np.float32), (w2 / s).astype(np.float32)


def _dispatch(i1, i2, w1, w2):
    """Per-expert token index lists and combine weights."""
    idx, cw = [], []
    for e in range(E):
        a = np.nonzero(i1 == e)[0]
        b = np.nonzero(i2 == e)[0]
        idx.append(np.concatenate([a, b]))
        cw.append(np.concatenate([w1[a], w2[b]]).astype(np.float32))
    return idx, cw


# ----------------------------------------------------------------------------
# Device program
# ----------------------------------------------------------------------------

def _chunks(cap):
    out, n0 = [], 0
    while n0 < cap:
        n = min(512, cap - n0)
        out.append((n0, n))
        n0 += n
    return out


def _build_program(cap):
    import concourse.bacc as bacc
    import concourse.mybir as mybir
    import concourse.tile as tile
    from concourse.bass import ds, ts
    from concourse._compat import get_trn_type
    from contextlib import ExitStack

    bf = mybir.dt.bfloat16
    f32 = mybir.dt.float32
    AF = mybir.ActivationFunctionType
    AX = mybir.AxisListType

    nc = bacc.Bacc(
        get_trn_type() or "TRN2",
        target_bir_lowering=False,
        debug=False,
        num_devices=NCORES,
    )

    def din(name, shape, dt):
        return nc.dram_tensor(name, shape, dt, kind="ExternalInput").ap()

    def dout(name, shape, dt):
        return nc.dram_tensor(name, shape, dt, kind="ExternalOutput").ap()

    xsh = din("xsh", [D, TSH], bf)            # x-shard, transposed [D, T]
    xg = din("xg", [NSLOT, D, cap], bf)       # dispatched tokens, transposed
    w1t = din("w1t", [D, F], bf)              # shared expert weights (pre-T)
    w3t = din("w3t", [D, F], bf)
    w2t = din("w2t", [F, D], bf)
    e1t = din("e1t", [NSLOT, D, F], bf)       # this core's 2 experts (pre-T)
    e3t = din("e3t", [NSLOT, D, F], bf)
    e2t = din("e2t", [NSLOT, F, D], bf)
    gwt = din("gwt", [D, E], bf)              # gate_w.T
    gb = din("gb", [128, E], f32)             # expert_bias replicated
    cwr = din("cwr", [NSLOT, 128, cap], f32)  # combine weights replicated

    probs = dout("probs", [TSH, E], f32)
    ysh = dout("ysh", [D, TSH], f32)          # shared-expert out, transposed
    rout = dout("rout", [NSLOT, D, cap], f32)  # routed out (cw-scaled), transposed

    KD = D // 128   # 8 contraction chunks for D
    KF = F // 128   # 4 contraction chunks for F
    FO = F // 128   # 4 output blocks for F
    DO = D // 128   # 8 output blocks for D

    with tile.TileContext(nc) as tc, ExitStack() as ctx:
        wp = ctx.enter_context(tc.tile_pool(name="wp", bufs=1))
        ep = ctx.enter_context(tc.tile_pool(name="ep", bufs=2))
        xp = ctx.enter_context(tc.tile_pool(name="xp", bufs=1))
        xgp = ctx.enter_context(tc.tile_pool(name="xgp", bufs=2))
        cwp = ctx.enter_context(tc.tile_pool(name="cwp", bufs=2))
        gp = ctx.enter_context(tc.tile_pool(name="gp", bufs=2))
        sp = ctx.enter_context(tc.tile_pool(name="sp", bufs=3))
        op = ctx.enter_context(tc.tile_pool(name="op", bufs=3))
        rp = ctx.enter_context(tc.tile_pool(name="rp", bufs=2))
        ps = ctx.enter_context(tc.tile_pool(name="ps", bufs=2, space="PSUM"))
        pr = ctx.enter_context(tc.tile_pool(name="pr", bufs=2, space="PSUM"))

        def sb_from(pool, dram2d, ko, tag):
            # [K*128, N] dram -> [128, K, N] sbuf tile
            t_ = pool.tile([128, ko, dram2d.shape[-1]], dram2d.dtype, tag=tag)
            nc.sync.dma_start(t_[:], dram2d.rearrange("(k p) n -> p k n", p=128))
            return t_

        # persistent operands
        xs = sb_from(xp, xsh, KD, "xs")
        gws = sb_from(wp, gwt, KD, "gws")
        gbs = wp.tile([128, E], f32, tag="gbs")
        nc.sync.dma_start(gbs[:], gb[:])
        w1s = sb_from(wp, w1t, KD, "w1s")
        w3s = sb_from(wp, w3t, KD, "w3s")
        w2s = sb_from(wp, w2t, KF, "w2s")

        # ------------------------- router -------------------------
        for tt in range(TSH // 128):
            plg = pr.tile([128, E], f32, tag="plg")
            for kd in range(KD):
                nc.tensor.matmul(
                    plg[:],
                    xs[:, kd, ts(tt, 128)],
                    gws[:, kd, :],
                    start=kd == 0,
                    stop=kd == KD - 1,
                )
            lgt = rp.tile([128, E], f32, tag="lgt")
            nc.vector.tensor_add(lgt[:], plg[:], gbs[:])
            mx = rp.tile([128, 1], f32, tag="mx")
            nc.vector.reduce_max(mx[:], lgt[:], axis=AX.X)
            lgs = rp.tile([128, E], f32, tag="lgs")
            nc.vector.tensor_scalar_sub(lgs[:], lgt[:], mx[:])
            ex = rp.tile([128, E], f32, tag="ex")
            nc.scalar.activation(ex[:], lgs[:], AF.Exp)
            sm = rp.tile([128, 1], f32, tag="sm")
            nc.vector.reduce_sum(sm[:], ex[:], axis=AX.X)
            rs = rp.tile([128, 1], f32, tag="rs")
            nc.vector.reciprocal(rs[:], sm[:])
            pb = rp.tile([128, E], f32, tag="pb")
            nc.vector.tensor_scalar_mul(pb[:], ex[:], rs[:])
            nc.sync.dma_start(probs[ts(tt, 128), :], pb[:])

        # ------------------------- SwiGLU FFN -------------------------
        def ffn(a1s, a3s, a2s, src, cw_sb, out_view, n0, ncols):
            g = gp.tile([128, KF, 512], bf, tag="g")
            for fo in range(FO):
                p1 = ps.tile([128, 512], f32, tag="p1")
                p3 = ps.tile([128, 512], f32, tag="p3")
                for kd in range(KD):
                    nc.tensor.matmul(
                        p1[:, :ncols],
                        a1s[:, kd, ts(fo, 128)],
                        src[:, kd, ds(n0, ncols)],
                        start=kd == 0,
                        stop=kd == KD - 1,
                    )
                for kd in range(KD):
                    nc.tensor.matmul(
                        p3[:, :ncols],
                        a3s[:, kd, ts(fo, 128)],
                        src[:, kd, ds(n0, ncols)],
                        start=kd == 0,
                        stop=kd == KD - 1,
                    )
                # silu(h1)*h3 = h1*sigmoid(h1)*h3 (Silu isn't in CoreSim)
                sg = sp.tile([128, 512], bf, tag="sg")
                nc.scalar.activation(sg[:, :ncols], p1[:, :ncols], AF.Sigmoid)
                sl = sp.tile([128, 512], bf, tag="sl")
                nc.vector.tensor_mul(sl[:, :ncols], p1[:, :ncols], sg[:, :ncols])
                nc.vector.tensor_mul(g[:, fo, :ncols], p3[:, :ncols], sl[:, :ncols])
                if cw_sb is not None:
                    nc.vector.tensor_mul(
                        g[:, fo, :ncols], g[:, fo, :ncols], cw_sb[:, ds(n0, ncols)]
                    )
            for do in range(DO):
                po = ps.tile([128, 512], f32, tag="po")
                for kf in range(KF):
                    nc.tensor.matmul(
                        po[:, :ncols],
                        a2s[:, kf, ts(do, 128)],
                        g[:, kf, :ncols],
                        start=kf == 0,
                        stop=kf == KF - 1,
                    )
                ob = op.tile([128, 512], f32, tag="ob")
                nc.vector.tensor_copy(ob[:, :ncols], po[:, :ncols])
                nc.sync.dma_start(out_view[:, do, ds(n0, ncols)], ob[:, :ncols])

        # shared expert over the token shard
        ysh_v = ysh.rearrange("(k p) n -> p k n", p=128)
        for n0, ncols in _chunks(TSH):
            ffn(w1s, w3s, w2s, xs, None, ysh_v, n0, ncols)

        # routed experts
        for s in range(NSLOT):
            a1s = sb_from(ep, e1t[s], KD, "a1s")
            a3s = sb_from(ep, e3t[s], KD, "a3s")
            a2s = sb_from(ep, e2t[s], KF, "a2s")
            xgs = sb_from(xgp, xg[s], KD, "xgs")
            cws = cwp.tile([128, cap], f32, tag="cws")
            nc.sync.dma_start(cws[:], cwr[s])
            rout_v = rout[s].rearrange("(k p) n -> p k n", p=128)
            for n0, ncols in _chunks(cap):
                ffn(a1s, a3s, a2s, xgs, cws, rout_v, n0, ncols)

    nc.compile()
    return nc


# ----------------------------------------------------------------------------
# Host orchestration
# ----------------------------------------------------------------------------

def _prepare(inputs):
    x = np.ascontiguousarray(np.asarray(inputs["x"], dtype=np.float32))
    gate_w = np.asarray(inputs["gate_w"], dtype=np.float32)
    expert_bias = np.asarray(inputs["expert_bias"], dtype=np.float32)
    sw1 = np.asarray(inputs["sw1"], dtype=np.float32)
    sw2 = np.asarray(inputs["sw2"], dtype=np.float32)
    sw3 = np.asarray(inputs["sw3"], dtype=np.float32)
    ew1 = np.asarray(inputs["ew1"], dtype=np.float32)
    ew2 = np.asarray(inputs["ew2"], dtype=np.float32)
    ew3 = np.asarray(inputs["ew3"], dtype=np.float32)

    flat_x = x.reshape(T, D)
    i1, i2, w1, w2 = _route(flat_x, gate_w, expert_bias)
    idx, cw = _dispatch(i1, i2, w1, w2)
    maxc = max(len(ix) for ix in idx)
    cap = max(256, ((maxc + 127) // 128) * 128)

    if cap not in _CACHE:
        _CACHE[cap] = _build_program(cap)
    nc = _CACHE[cap]

    xT = np.ascontiguousarray(flat_x.T)                      # [D, T] fp32
    xT_bf = xT.astype(BF16)

    # shared weights, pre-transposed, bf16
    w1t = np.ascontiguousarray(sw1[0].T).astype(BF16)        # [D, F]
    w3t = np.ascontiguousarray(sw3[0].T).astype(BF16)
    w2t = np.ascontiguousarray(sw2[0].T).astype(BF16)        # [F, D]
    gwt = np.ascontiguousarray(gate_w.T).astype(BF16)        # [D, E]
    gb = np.ascontiguousarray(
        np.broadcast_to(expert_bias[None, :], (128, E))
    ).astype(np.float32)

    in_maps = []
    for c in range(NCORES):
        es = [2 * c, 2 * c + 1]
        xg = np.zeros((NSLOT, D, cap), dtype=BF16)
        cwr = np.zeros((NSLOT, 128, cap), dtype=np.float32)
        for s, e in enumerate(es):
            n = len(idx[e])
            # gather token columns from the transposed activations
            xg[s, :, :n] = xT_bf[:, idx[e]]
            cwr[s, :, :n] = cw[e][None, :]
        in_maps.append(
            {
                "xsh": np.ascontiguousarray(xT_bf[:, c * TSH : (c + 1) * TSH]),
                "xg": xg,
                "w1t": w1t,
                "w3t": w3t,
                "w2t": w2t,
                "e1t": np.ascontiguousarray(ew1[es].transpose(0, 2, 1)).astype(BF16),
                "e3t": np.ascontiguousarray(ew3[es].transpose(0, 2, 1)).astype(BF16),
                "e2t": np.ascontiguousarray(ew2[es].transpose(0, 2, 1)).astype(BF16),
                "gwt": gwt,
                "gb": gb,
                "cwr": cwr,
            }
        )
    meta = {"idx": idx, "cap": cap}
    return nc, in_maps, meta


def _postprocess(results, meta):
    idx = meta["idx"]
    y = np.empty((T, D), dtype=np.float32)
    for c in range(NCORES):
        y[c * TSH : (c + 1) * TSH] = results[c]["ysh"].T
    for e in range(E):
        c, s = e // 2, e % 2
        n = len(idx[e])
        if n:
            y[idx[e]] += results[c]["rout"][s][:, :n].T
    probs = np.concatenate([results[c]["probs"] for c in range(NCORES)], axis=0)
    return y.reshape(B, L, D), probs


def kernel(**inputs):
    from concourse.bass_utils import run_bass_kernel_spmd

    nc, in_maps, meta = _prepare(inputs)
    res = run_bass_kernel_spmd(nc, in_maps, list(range(NCORES)))
    return _postprocess(res.results, meta)
